# revision 12
# baseline (speedup 1.0000x reference)
"""MoE experts forward (dispatch -> per-expert SwiGLU FFN -> weighted combine)
on 8 Trainium2 NeuronCores, expert-parallel: one expert per core.

Host side: route tokens to experts (merge duplicate top-k hits of the same
expert via summed weights), pad each expert's token set to a common capacity C,
and lay tensors out in the [partition, outer, free] form the TensorEngine
wants. Device side (SPMD, one program on 8 cores): a fused
x @ W1 -> SwiGLU -> @ W2 pipeline in float32r (full-rate fp32 matmul mode),
keeping the token block and the intermediate activations resident in SBUF so
each weight byte is streamed from HBM exactly once. Host side again: scatter
the per-expert outputs back with the routing weights.

Problem shape (hardcoded): N=4096 tokens, H=2048, I=1408, E=8 experts, top-2.
"""

import numpy as np

P = 128
H = 2048
I = 1408
E = 8
KH = H // P          # 16 k-tiles for the gate/up matmul
KI = I // P          # 11 k-tiles for the down matmul
MI = (2 * I) // P    # 22 m-tiles of the fused gate_up output
MG = I // P          # 11 gate m-tiles (up tile j pairs with gate tile j)
MH = H // P          # 16 m-tiles of the down output
NTS = 512            # token tile (matmul free dim; fp32 moving-operand max)

_cache = {}


def _build(C):
    """Build + compile the 8-core SPMD program for token capacity C."""
    import concourse.mybir as mybir
    import concourse.tile as tile
    from concourse import bacc
    from concourse.bass import ts, _add_dep_helper

    f32 = mybir.dt.float32
    f32r = mybir.dt.float32r
    silu = mybir.ActivationFunctionType.Silu
    NN = C // NTS
    MH2 = MH // 2

    nc = bacc.Bacc("TRN2", target_bir_lowering=False, debug=False, num_devices=E)

    xT = nc.dram_tensor("xT", [P, KH, C], f32r, kind="ExternalInput")
    # w1: gate/up m-tile pair j packed together -> one DMA per j
    w1 = nc.dram_tensor("w1", [P, MI, KH, P], f32r, kind="ExternalInput")
    # w2: adjacent output m-tile pairs packed together
    w2 = nc.dram_tensor("w2", [P, MH, KI, P], f32r, kind="ExternalInput")
    yT = nc.dram_tensor("yT", [P, MH, C], f32, kind="ExternalOutput")

    with tile.TileContext(nc) as tc:
        with (
            tc.tile_pool(name="xp", bufs=1) as xp,
            tc.tile_pool(name="hp", bufs=1) as hp,
            tc.tile_pool(name="w1p", bufs=6) as w1p,
            tc.tile_pool(name="w2p", bufs=3) as w2p,
            tc.tile_pool(name="yp", bufs=2) as yp,
            tc.tile_pool(name="sp", bufs=3) as sp,
            tc.tile_pool(name="ps", bufs=8, space="PSUM") as ps,
        ):
            x_sb = [None] * KH
            x_dmas = []
            w1_tiles = {}
            w1_dmas = {}
            silus = {}

            def load_x(k, eng):
                t = xp.tile([P, C], f32r, tag=f"x{k}", name=f"x{k}")
                x_dmas.append(eng.dma_start(t[:], xT[:, k, :]))
                x_sb[k] = t

            def load_w1(j, eng, anchor=None):
                wg = w1p.tile([P, KH, P], f32r, tag="w1", name=f"wg{j}")
                dg = eng.dma_start(wg[:], w1[:, j])
                wu = w1p.tile([P, KH, P], f32r, tag="w1", name=f"wu{j}")
                du = eng.dma_start(wu[:], w1[:, j + MG])
                w1_tiles[j] = (wg, wu)
                w1_dmas[j] = (dg, du)
                if anchor is not None:
                    # same-engine ordering only: paces prefetch ~2 j's behind
                    # compute so it never competes with critical transfers
                    _add_dep_helper(dg.ins, anchor.ins, sync=False, reason="w1 pacing")
                    _add_dep_helper(du.ins, anchor.ins, sync=False, reason="w1 pacing")

            def evict(j, n, pg, pu):
                nsl = ts(n, NTS)
                sg = sp.tile([P, NTS], f32, tag="sg", name=f"sg{j}n{n}")
                silus[(j, n)] = nc.scalar.activation(sg[:], pg[:], silu)
                nc.vector.tensor_mul(h_sb[j][:, nsl], sg[:], pu[:])

            # Stripe the critical prologue payload across both HWDGE queues,
            # ordered by when the k-outer matmul block needs each piece:
            # x0 + gate weights on sync, up weights + early x on scalar.
            wg0 = w1p.tile([P, KH, P], f32r, tag="w1", name="wg0")
            wu0 = w1p.tile([P, KH, P], f32r, tag="w1", name="wu0")
            wg1 = w1p.tile([P, KH, P], f32r, tag="w1", name="wg1")
            wu1 = w1p.tile([P, KH, P], f32r, tag="w1", name="wu1")
            load_x(0, nc.sync)
            nc.scalar.dma_start(wu0[:], w1[:, 0 + MG])
            nc.sync.dma_start(wg0[:], w1[:, 0])
            nc.scalar.dma_start(wg1[:], w1[:, 1])
            load_x(1, nc.sync)
            nc.scalar.dma_start(wu1[:], w1[:, 1 + MG])
            w1_tiles[0] = (wg0, wu0)
            w1_tiles[1] = (wg1, wu1)
            for k in (3, 5, 7, 9, 11, 13, 15):
                load_x(k, nc.sync)
            for k in (2, 4, 6, 8, 10, 12, 14):
                load_x(k, nc.scalar)
            load_w1(2, nc.sync)
            load_w1(3, nc.scalar)

            h_sb = [hp.tile([P, C], f32r, tag=f"h{j}", name=f"h{j}") for j in range(MG)]

            # Phase 1: gu = x @ W1, h = silu(gate) * up, kept in SBUF.
            j_start = 0
            if NN == 2:
                # j=0,1 emitted k-outer across all 8 PSUM banks: each
                # arriving x chunk unlocks 8 matmuls, keeping the PE busy
                # while the token block streams in.
                groups = []
                evs = []
                for j in (0, 1):
                    wg, wu = w1_tiles[j]
                    pp = {}
                    for n in range(NN):
                        pg = ps.tile([P, NTS], f32, tag="ps", name=f"pg{j}n{n}")
                        pu = ps.tile([P, NTS], f32, tag="ps", name=f"pu{j}n{n}")
                        pp[n] = (pg, pu)
                        evs.append((j, n, pg, pu))
                    for n in range(NN):
                        groups.append((pp[n][0], wg, n))
                    for n in range(NN):
                        groups.append((pp[n][1], wu, n))
                for k in range(KH):
                    for pt, wt, n in groups:
                        nc.tensor.matmul(
                            pt[:], wt[:, k, :], x_sb[k][:, ts(n, NTS)],
                            start=(k == 0), stop=(k == KH - 1),
                        )
                for j, n, pg, pu in evs:
                    evict(j, n, pg, pu)
                j_start = 2

            for j in range(j_start, MG):
                if j not in w1_tiles:
                    load_w1(j, nc.scalar, anchor=silus[(j - 3, NN - 1)])
                wg, wu = w1_tiles[j]
                for n in range(NN):
                    nsl = ts(n, NTS)
                    pg = ps.tile([P, NTS], f32, tag="ps", name=f"pgs{j}n{n}")
                    pu = ps.tile([P, NTS], f32, tag="ps", name=f"pus{j}n{n}")
                    for k in range(KH):
                        nc.tensor.matmul(
                            pg[:], wg[:, k, :], x_sb[k][:, nsl],
                            start=(k == 0), stop=(k == KH - 1),
                        )
                    for k in range(KH):
                        nc.tensor.matmul(
                            pu[:], wu[:, k, :], x_sb[k][:, nsl],
                            start=(k == 0), stop=(k == KH - 1),
                        )
                    evict(j, n, pg, pu)

            # Phase 2: y = h @ W2.
            for m in range(MH):
                wd = w2p.tile([P, KI, P], f32r, tag="w2", name=f"wd{m}")
                dw = nc.scalar.dma_start(wd[:], w2[:, m])
                _add_dep_helper(
                    dw.ins, silus[(min(2 + m, MG - 1), NN - 1)].ins, sync=False,
                    reason="w2 prefetch pacing",
                )
                for n in range(NN):
                    nsl = ts(n, NTS)
                    py = ps.tile([P, NTS], f32, tag="ps", name=f"py{m}n{n}")
                    for k in range(KI):
                        nc.tensor.matmul(
                            py[:], wd[:, k, :], h_sb[k][:, nsl],
                            start=(k == 0), stop=(k == KI - 1),
                        )
                    y_sb = yp.tile([P, NTS], f32, tag="y", name=f"y{m}n{n}")
                    nc.vector.tensor_copy(y_sb[:], py[:])
                    nc.sync.dma_start(yT[:, m, nsl], y_sb[:])

    nc.compile()
    return nc


def _round_f32r(a):
    """Round fp32 -> fp32r (e8m11: round-to-nearest-even, low 12 bits zero)."""
    b = np.ascontiguousarray(a, dtype=np.float32).view(np.uint32)
    lsb = (b >> 12) & 1
    out = (b + 0x7FF + lsb) & np.uint32(0xFFFFF000)
    return out.view(np.float32)


def _route(routing_weights, selected_experts, n_tok):
    """Merged combine weights [E, N] and per-expert token index lists."""
    comb = np.zeros((E, n_tok), dtype=np.float64)
    np.add.at(
        comb,
        (selected_experts, np.arange(n_tok)[:, None]),
        routing_weights.astype(np.float64),
    )
    comb = comb.astype(np.float32)
    idx = [np.nonzero(comb[e] != 0)[0] for e in range(E)]
    return comb, idx


def prepare(inputs):
    """Route + pad + lay out per-core input maps. Returns (C, in_maps, comb, idx)."""
    hs = _round_f32r(inputs["hidden_states"])
    rw = np.asarray(inputs["routing_weights"], dtype=np.float32)
    sel = np.asarray(inputs["selected_experts"]).astype(np.int64)
    gup = _round_f32r(inputs["gate_up_proj"])
    dwn = _round_f32r(inputs["down_proj"])
    n_tok = hs.shape[0]

    comb, idx = _route(rw, sel, n_tok)
    maxc = max(int(len(ix)) for ix in idx)
    C = max(1024, -(-maxc // NTS) * NTS)

    in_maps = []
    for e in range(E):
        ix = idx[e]
        xg = np.zeros((C, H), dtype=np.float32)
        xg[: len(ix)] = hs[ix]
        # [C,H] -> [P, KH, C] with row h = ko*P + p
        xT = np.ascontiguousarray(xg.T.reshape(KH, P, C).transpose(1, 0, 2))
        # [H, 2I] -> [P, MI, KH, P]: w1[p, m, ko, j] = W1[ko*P+p, m*P+j]
        w1 = np.ascontiguousarray(
            gup[e].reshape(KH, P, MI, P).transpose(1, 2, 0, 3)
        )
        # [I, H] -> [P, MH, KI, P]
        w2 = np.ascontiguousarray(
            dwn[e].reshape(KI, P, MH, P).transpose(1, 2, 0, 3)
        )
        in_maps.append({"xT": xT, "w1": w1, "w2": w2})
    return C, in_maps, comb, idx


def combine(res_list, C, comb, idx, n_tok):
    """Scatter per-expert outputs back into the full [N, H] output."""
    out = np.zeros((n_tok, H), dtype=np.float32)
    for e in range(E):
        ix = idx[e]
        yT = res_list[e]["yT"]  # [P, MH, C]
        y = yT.transpose(2, 1, 0).reshape(C, H)[: len(ix)]
        out[ix] += comb[e, ix][:, None] * y
    return out


def kernel(**inputs):
    from concourse.bass_utils import run_bass_kernel_spmd

    n_tok = inputs["hidden_states"].shape[0]
    C, in_maps, comb, idx = prepare(inputs)
    if C not in _cache:
        _cache[C] = _build(C)
    res = run_bass_kernel_spmd(_cache[C], in_maps, core_ids=list(range(E)))
    return combine(res.results, C, comb, idx, n_tok)


# revision 15
# speedup vs baseline: 1.0169x; 1.0169x over previous
"""MoE experts forward (dispatch -> per-expert SwiGLU FFN -> weighted combine)
on 8 Trainium2 NeuronCores, expert-parallel: one expert per core.

Host side: route tokens to experts (merge duplicate top-k hits of the same
expert via summed weights), pad each expert's token set to a common capacity C,
and lay tensors out in the [partition, outer, free] form the TensorEngine
wants. Device side (SPMD, one program on 8 cores): a fused
x @ W1 -> SwiGLU -> @ W2 pipeline in float32r (full-rate fp32 matmul mode),
keeping the token block and the intermediate activations resident in SBUF so
each weight byte is streamed from HBM exactly once. Host side again: scatter
the per-expert outputs back with the routing weights.

Problem shape (hardcoded): N=4096 tokens, H=2048, I=1408, E=8 experts, top-2.
"""

import numpy as np

P = 128
H = 2048
I = 1408
E = 8
KH = H // P          # 16 k-tiles for the gate/up matmul
KI = I // P          # 11 k-tiles for the down matmul
MI = (2 * I) // P    # 22 m-tiles of the fused gate_up output
MG = I // P          # 11 gate m-tiles (up tile j pairs with gate tile j)
MH = H // P          # 16 m-tiles of the down output
NTS = 512            # token tile (matmul free dim; fp32 moving-operand max)

_cache = {}


def _build(C):
    """Build + compile the 8-core SPMD program for token capacity C."""
    import concourse.mybir as mybir
    import concourse.tile as tile
    from concourse import bacc
    from concourse.bass import ts, _add_dep_helper

    f32 = mybir.dt.float32
    f32r = mybir.dt.float32r
    silu = mybir.ActivationFunctionType.Silu
    NN = C // NTS
    MH2 = MH // 2

    nc = bacc.Bacc("TRN2", target_bir_lowering=False, debug=False, num_devices=E)

    xT = nc.dram_tensor("xT", [P, KH, C], f32r, kind="ExternalInput")
    # w1: gate/up m-tile pair j packed together -> one DMA per j
    w1 = nc.dram_tensor("w1", [P, MI, KH, P], f32r, kind="ExternalInput")
    # w2: adjacent output m-tile pairs packed together
    w2 = nc.dram_tensor("w2", [P, MH, KI, P], f32r, kind="ExternalInput")
    yT = nc.dram_tensor("yT", [P, MH, C], f32, kind="ExternalOutput")

    w1_bufs = 6 if NN <= 2 else 2
    w2_bufs = 3 if NN <= 2 else 1
    with tile.TileContext(nc) as tc:
        with (
            tc.tile_pool(name="xp", bufs=1) as xp,
            tc.tile_pool(name="hp", bufs=1) as hp,
            tc.tile_pool(name="w1p", bufs=w1_bufs) as w1p,
            tc.tile_pool(name="w2p", bufs=w2_bufs) as w2p,
            tc.tile_pool(name="yp", bufs=2) as yp,
            tc.tile_pool(name="sp", bufs=3) as sp,
            tc.tile_pool(name="ps", bufs=8, space="PSUM") as ps,
        ):
            x_sb = [None] * KH
            x_dmas = []
            w1_tiles = {}
            w1_dmas = {}
            silus = {}

            def load_x(k, eng):
                t = xp.tile([P, C], f32r, tag=f"x{k}", name=f"x{k}")
                x_dmas.append(eng.dma_start(t[:], xT[:, k, :]))
                x_sb[k] = t

            def load_w1(j, eng, anchor=None):
                wg = w1p.tile([P, KH, P], f32r, tag="w1", name=f"wg{j}")
                dg = eng.dma_start(wg[:], w1[:, j])
                wu = w1p.tile([P, KH, P], f32r, tag="w1", name=f"wu{j}")
                du = eng.dma_start(wu[:], w1[:, j + MG])
                w1_tiles[j] = (wg, wu)
                w1_dmas[j] = (dg, du)
                if anchor is not None:
                    # same-engine ordering only: paces prefetch ~2 j's behind
                    # compute so it never competes with critical transfers
                    _add_dep_helper(dg.ins, anchor.ins, sync=False, reason="w1 pacing")
                    _add_dep_helper(du.ins, anchor.ins, sync=False, reason="w1 pacing")

            def evict(j, n, pg, pu):
                nsl = ts(n, NTS)
                sg = sp.tile([P, NTS], f32, tag="sg", name=f"sg{j}n{n}")
                silus[(j, n)] = nc.scalar.activation(sg[:], pg[:], silu)
                nc.vector.tensor_mul(h_sb[j][:, nsl], sg[:], pu[:])

            # Stripe the critical prologue payload across both HWDGE queues,
            # ordered by when the k-outer matmul block needs each piece:
            # x0 + gate weights on sync, up weights + early x on scalar.
            if NN == 2:
                wg0 = w1p.tile([P, KH, P], f32r, tag="w1", name="wg0")
                wu0 = w1p.tile([P, KH, P], f32r, tag="w1", name="wu0")
                wg1 = w1p.tile([P, KH, P], f32r, tag="w1", name="wg1")
                wu1 = w1p.tile([P, KH, P], f32r, tag="w1", name="wu1")
                load_x(0, nc.sync)
                nc.scalar.dma_start(wu0[:], w1[:, 0 + MG])
                nc.sync.dma_start(wg0[:], w1[:, 0])
                nc.scalar.dma_start(wg1[:], w1[:, 1])
                load_x(1, nc.sync)
                nc.scalar.dma_start(wu1[:], w1[:, 1 + MG])
                w1_tiles[0] = (wg0, wu0)
                w1_tiles[1] = (wg1, wu1)
                for k in (3, 5, 7, 9, 11, 13, 15):
                    load_x(k, nc.sync)
                for k in (2, 4, 6, 8, 10, 12, 14):
                    load_x(k, nc.scalar)
                load_w1(2, nc.sync)
                load_w1(3, nc.scalar)
            else:
                for k in range(KH):
                    load_x(k, nc.sync if k % 2 == 0 else nc.scalar)

            h_sb = [hp.tile([P, C], f32r, tag=f"h{j}", name=f"h{j}") for j in range(MG)]

            # Phase 1: gu = x @ W1, h = silu(gate) * up, kept in SBUF.
            j_start = 0
            if NN == 2:
                # j=0,1 emitted k-outer across all 8 PSUM banks: each
                # arriving x chunk unlocks 8 matmuls, keeping the PE busy
                # while the token block streams in.
                groups = []
                evs = []
                for j in (0, 1):
                    wg, wu = w1_tiles[j]
                    pp = {}
                    for n in range(NN):
                        pg = ps.tile([P, NTS], f32, tag="ps", name=f"pg{j}n{n}")
                        pu = ps.tile([P, NTS], f32, tag="ps", name=f"pu{j}n{n}")
                        pp[n] = (pg, pu)
                        evs.append((j, n, pg, pu))
                    for n in range(NN):
                        groups.append((pp[n][0], wg, n))
                    for n in range(NN):
                        groups.append((pp[n][1], wu, n))
                for k in range(KH):
                    for pt, wt, n in groups:
                        nc.tensor.matmul(
                            pt[:], wt[:, k, :], x_sb[k][:, ts(n, NTS)],
                            start=(k == 0), stop=(k == KH - 1),
                        )
                for j, n, pg, pu in evs:
                    evict(j, n, pg, pu)
                j_start = 2

            for j in range(j_start, MG):
                if j not in w1_tiles:
                    load_w1(j, nc.scalar, anchor=silus.get((j - 3, NN - 1)))
                wg, wu = w1_tiles[j]
                for n in range(NN):
                    nsl = ts(n, NTS)
                    pg = ps.tile([P, NTS], f32, tag="ps", name=f"pgs{j}n{n}")
                    pu = ps.tile([P, NTS], f32, tag="ps", name=f"pus{j}n{n}")
                    for k in range(KH):
                        nc.tensor.matmul(
                            pg[:], wg[:, k, :], x_sb[k][:, nsl],
                            start=(k == 0), stop=(k == KH - 1),
                        )
                    for k in range(KH):
                        nc.tensor.matmul(
                            pu[:], wu[:, k, :], x_sb[k][:, nsl],
                            start=(k == 0), stop=(k == KH - 1),
                        )
                    evict(j, n, pg, pu)

            # Phase 2: y = h @ W2.
            for m in range(MH):
                wd = w2p.tile([P, KI, P], f32r, tag="w2", name=f"wd{m}")
                dw = nc.scalar.dma_start(wd[:], w2[:, m])
                _add_dep_helper(
                    dw.ins, silus[(min(2 + m, MG - 1), NN - 1)].ins, sync=False,
                    reason="w2 prefetch pacing",
                )
                for n in range(NN):
                    nsl = ts(n, NTS)
                    py = ps.tile([P, NTS], f32, tag="ps", name=f"py{m}n{n}")
                    for k in range(KI):
                        nc.tensor.matmul(
                            py[:], wd[:, k, :], h_sb[k][:, nsl],
                            start=(k == 0), stop=(k == KI - 1),
                        )
                    y_sb = yp.tile([P, NTS], f32, tag="y", name=f"y{m}n{n}")
                    nc.vector.tensor_copy(y_sb[:], py[:])
                    nc.sync.dma_start(yT[:, m, nsl], y_sb[:])

    nc.compile()
    return nc


def _round_f32r(a):
    """Round fp32 -> fp32r (e8m11: round-to-nearest-even, low 12 bits zero)."""
    b = np.ascontiguousarray(a, dtype=np.float32).view(np.uint32)
    lsb = (b >> 12) & 1
    out = (b + 0x7FF + lsb) & np.uint32(0xFFFFF000)
    return out.view(np.float32)


def _route(routing_weights, selected_experts, n_tok):
    """Merged combine weights [E, N] and per-expert token index lists."""
    comb = np.zeros((E, n_tok), dtype=np.float64)
    np.add.at(
        comb,
        (selected_experts, np.arange(n_tok)[:, None]),
        routing_weights.astype(np.float64),
    )
    comb = comb.astype(np.float32)
    idx = [np.nonzero(comb[e] != 0)[0] for e in range(E)]
    return comb, idx


def prepare(inputs):
    """Route + pad + lay out per-core input maps. Returns (C, in_maps, comb, idx)."""
    hs = _round_f32r(inputs["hidden_states"])
    rw = np.asarray(inputs["routing_weights"], dtype=np.float32)
    sel = np.asarray(inputs["selected_experts"]).astype(np.int64)
    gup = _round_f32r(inputs["gate_up_proj"])
    dwn = _round_f32r(inputs["down_proj"])
    n_tok = hs.shape[0]

    comb, idx = _route(rw, sel, n_tok)
    maxc = max(int(len(ix)) for ix in idx)
    C = max(1024, -(-maxc // NTS) * NTS)

    in_maps = []
    for e in range(E):
        ix = idx[e]
        xg = np.zeros((C, H), dtype=np.float32)
        xg[: len(ix)] = hs[ix]
        # [C,H] -> [P, KH, C] with row h = ko*P + p
        xT = np.ascontiguousarray(xg.T.reshape(KH, P, C).transpose(1, 0, 2))
        # [H, 2I] -> [P, MI, KH, P]: w1[p, m, ko, j] = W1[ko*P+p, m*P+j]
        w1 = np.ascontiguousarray(
            gup[e].reshape(KH, P, MI, P).transpose(1, 2, 0, 3)
        )
        # [I, H] -> [P, MH, KI, P]
        w2 = np.ascontiguousarray(
            dwn[e].reshape(KI, P, MH, P).transpose(1, 2, 0, 3)
        )
        in_maps.append({"xT": xT, "w1": w1, "w2": w2})
    return C, in_maps, comb, idx


def combine(res_list, C, comb, idx, n_tok):
    """Scatter per-expert outputs back into the full [N, H] output."""
    out = np.zeros((n_tok, H), dtype=np.float32)
    for e in range(E):
        ix = idx[e]
        yT = res_list[e]["yT"]  # [P, MH, C]
        y = yT.transpose(2, 1, 0).reshape(C, H)[: len(ix)]
        out[ix] += comb[e, ix][:, None] * y
    return out


def _kernel_numpy(inputs, comb, idx, n_tok):
    """Host fallback for routing so imbalanced it exceeds SBUF capacity."""
    hs = np.asarray(inputs["hidden_states"], dtype=np.float32)
    gup = np.asarray(inputs["gate_up_proj"], dtype=np.float32)
    dwn = np.asarray(inputs["down_proj"], dtype=np.float32)
    out = np.zeros((n_tok, H), dtype=np.float32)
    for e in range(E):
        ix = idx[e]
        if len(ix) == 0:
            continue
        gu = hs[ix] @ gup[e]
        gate, up = gu[:, :I], gu[:, I:]
        y = (gate / (1.0 + np.exp(-gate)) * up) @ dwn[e]
        out[ix] += comb[e, ix][:, None] * y
    return out


def kernel(**inputs):
    from concourse.bass_utils import run_bass_kernel_spmd

    n_tok = inputs["hidden_states"].shape[0]
    C, in_maps, comb, idx = prepare(inputs)
    if C > 1536:
        return _kernel_numpy(inputs, comb, idx, n_tok)
    if C not in _cache:
        _cache[C] = _build(C)
    res = run_bass_kernel_spmd(_cache[C], in_maps, core_ids=list(range(E)))
    return combine(res.results, C, comb, idx, n_tok)
